# revision 2
# baseline (speedup 1.0000x reference)
"""Trainium2 Bass kernel for the KB criterion loss.

Math
----
reference:
    diff[b,i,j] = probs[b,j] - probs[b,i]
    loss = sum_ij mean_b (diff^2 * C[i,j]) / (n_pos + 1e-8),  n_pos = count(C > 0)

Expanding the square removes the [B,N,N] intermediate entirely:

    sum_b (P[b,i] - P[b,j])^2 = S2_i + S2_j - 2*G_ij
        with S2_j = sum_b P[b,j]^2   and   G = P^T P  (Gram matrix)

so   total = sum_ij C_ij * D_ij,   D = S2_i + S2_j - 2 G_ij
     loss  = (total / B) / (n_pos + 1e-8)

Sharding (8 cores)
------------------
Shard C by rows: core k owns rows S_k = [128k, 128k+128). P is replicated.
Each core moves 0.5MB of C + 0.5MB of P (vs 4MB of C with batch-parallel
sharding). Inputs are column-rolled by 128k so every core runs the same
program with its own row block mapped to local columns [0:128).

Per-core pipeline:
  1. DMA P (full, rolled) and C (row block, rolled) to SBUF.
  2. DVE: Psq = P*P.
  3. PE:  S2h = -(1/2) * ones^T @ Psq  (contract over b)  -> [1, N] (2 matmuls)
  4. ACT: copy S2h PSUM->SBUF.
  5. PE:  D' accumulated in PSUM per 512-col bank:
            D' = P_Sk^T @ P  (Gram block, contract over b=128 partitions)
               + S2h_Sk ⊗ 1  (rank-1, K=1)
               + 1 ⊗ S2h     (rank-1, K=1)
          so D' = G - S2_i/2 - S2_j/2 = -D/2.
  6. DVE: tensor_tensor_reduce: (C * D') * (-2), accum per partition -> [128,1].
  7. ACT: Sign(C) with accum_out -> per-partition n_pos counts [128,1].
  8. PE:  ones^T reduction of both columns -> two scalars; DMA [1,2] out.

Host sums the 8 partial pairs (the scalar all-reduce) and finishes the
division.
"""

import numpy as np

import concourse.bass as bass
import concourse.tile as tile
from concourse import mybir
from concourse.bass_utils import run_bass_kernel_spmd

B = 128
N = 1024
NCORES = 8
SH = N // NCORES  # 128 rows of C per core
F32 = mybir.dt.float32
HALF = 512  # fp32 moving-operand max free dim per matmul / PSUM bank


def build_bass() -> bass.Bass:
    nc = bass.Bass()
    p_d = nc.dram_tensor("probs_r", [B, N], F32, kind="ExternalInput")
    c_d = nc.dram_tensor("co_r", [SH, N], F32, kind="ExternalInput")
    o_d = nc.dram_tensor("out", [1, 2], F32, kind="ExternalOutput")

    with tile.TileContext(nc) as tc:
        with (
            tc.tile_pool(name="sb", bufs=1) as sb,
            tc.tile_pool(name="ps", bufs=1, space="PSUM") as ps,
        ):
            p_sb = sb.tile([B, N], F32)
            c_sb = sb.tile([SH, N], F32)
            psq = sb.tile([B, N], F32)
            s2h = sb.tile([1, N], F32)
            ones_row = sb.tile([1, HALF], F32)
            nh_col = sb.tile([B, 1], F32)
            ones_col_act = sb.tile([B, 1], F32)
            pcol = sb.tile([SH, 1], F32)
            npcol = sb.tile([SH, 1], F32)
            scr0 = sb.tile([SH, N], F32)
            scr1 = sb.tile([SH, N], F32)
            scr2 = sb.tile([SH, N], F32)
            out_sb = sb.tile([1, 2], F32)

            d_ps = ps.tile([B, N], F32)  # 2 banks
            s2_ps0 = ps.tile([1, HALF], F32)
            s2_ps1 = ps.tile([1, HALF], F32)
            fin0 = ps.tile([1, 1], F32)
            fin1 = ps.tile([1, 1], F32)

            # trn2 LDWEIGHTS carries ONE sync-wait slot, so each matmul's
            # operands must trace back to a single upstream engine.
            # Constants are therefore born on the engine their consumer
            # pairs with: DVE consts pair with DVE-produced psq; ACT
            # consts pair with ACT-produced s2h / pcol / npcol.
            nc.vector.memset(nh_col, -0.5)
            # ACT-born ones: Copy(x*0 + 1) — input never contributes
            nc.scalar.activation(
                ones_row, ones_row, mybir.ActivationFunctionType.Copy,
                bias=1.0, scale=0.0,
            )
            nc.scalar.activation(
                ones_col_act, ones_col_act, mybir.ActivationFunctionType.Copy,
                bias=1.0, scale=0.0,
            )

            # loads; P first (it heads the S2 critical path). One dma_start
            # per input measured faster than split-queue variants (29.8 vs
            # 31.1 us): queues share the 16 SDMA engines and extra DMA
            # semaphores cost wait-split NOP stalls.
            nc.sync.dma_start(out=p_sb, in_=p_d[:, :])
            nc.sync.dma_start(out=c_sb, in_=c_d[:, :])

            # Psq = P*P
            nc.vector.tensor_mul(psq, p_sb, p_sb)

            # -S2/2 = (-1/2)·colsum_b(Psq) -> [1, N] in PSUM, then to SBUF
            nc.tensor.matmul(s2_ps0, nh_col, psq[:, 0:HALF], start=True, stop=True)
            nc.tensor.matmul(s2_ps1, nh_col, psq[:, HALF:N], start=True, stop=True)
            nc.scalar.copy(s2h[0:1, 0:HALF], s2_ps0)
            nc.scalar.copy(s2h[0:1, HALF:N], s2_ps1)

            # D' = G - S2_i/2 - S2_j/2  (= -D/2) accumulated per PSUM bank.
            # lhsT = p_sb slice (same DMA sem as rhs -> one wait).
            for h in range(2):
                js = slice(HALF * h, HALF * (h + 1))
                nc.tensor.matmul(
                    d_ps[:, js], p_sb[:, 0:SH], p_sb[:, js], start=True, stop=False
                )
                nc.tensor.matmul(
                    d_ps[:, js], s2h[0:1, 0:SH], ones_row[0:1, :],
                    start=False, stop=False,
                )
                nc.tensor.matmul(
                    d_ps[:, js], ones_row[0:1, 0:SH], s2h[0:1, js],
                    start=False, stop=True,
                )

            # scr0 = C * D' on DVE; ACT reduce applies the -2 (scale imm):
            # pcol = sum_j -2*C*D' = sum_j C*D
            nc.vector.tensor_mul(scr0, c_sb, d_ps)
            nc.scalar.activation(
                scr1, scr0, mybir.ActivationFunctionType.Copy,
                scale=-2.0, accum_out=pcol,
            )

            # n_pos per partition: sum_j sign(C)  (C >= 0 always)
            nc.scalar.activation(
                scr2, c_sb, mybir.ActivationFunctionType.Sign, accum_out=npcol
            )

            # partition reduce -> scalars (all-ACT operand pairs)
            nc.tensor.matmul(fin0, ones_col_act, pcol, start=True, stop=True)
            nc.tensor.matmul(fin1, ones_col_act, npcol, start=True, stop=True)
            nc.scalar.copy(out_sb[0:1, 0:1], fin0)
            nc.scalar.copy(out_sb[0:1, 1:2], fin1)

            nc.sync.dma_start(out=o_d[:, :], in_=out_sb)

    _split_multi_waits(nc)
    return nc


def _split_multi_waits(nc: bass.Bass):
    """This walrus build accepts only ONE sync-wait per instruction
    ("Too many sync wait commands"). Tile's kernel-tail drain carries one
    wait per live semaphore; peel the extras onto same-engine NOPs that
    each stall on a single semaphore — semantically identical."""
    for bb in nc.main_func.blocks:
        insts = bb.instructions
        i = 0
        while i < len(insts):
            ins = insts[i]
            si = getattr(ins, "sync_info", None)
            if si is not None and si.on_wait is not None and len(si.on_wait) > 1:
                waits = list(si.on_wait)
                nops = []
                for j, w in enumerate(waits[:-1]):
                    nop = mybir.InstNoOp(
                        name=f"{ins.name}-wsplit{j}",
                        sync_info=mybir.SyncInfo(on_wait=[w], on_update=[]),
                        bass_nofuse=True,
                        engine=ins.engine,
                    )
                    nc.register_instruction(nop, overwrite=True)
                    nops.append(nop)
                si.on_wait = [waits[-1]]
                insts[i:i] = nops
                i += len(nops)
            i += 1


_NC = None


def _get_nc() -> bass.Bass:
    global _NC
    if _NC is None:
        _NC = build_bass()
    return _NC


def make_in_maps(probs: np.ndarray, co_matrix: np.ndarray):
    probs = np.ascontiguousarray(np.asarray(probs, dtype=np.float32))
    co_matrix = np.ascontiguousarray(np.asarray(co_matrix, dtype=np.float32))
    in_maps = []
    for k in range(NCORES):
        shift = -SH * k
        p_r = np.ascontiguousarray(np.roll(probs, shift, axis=1))
        c_r = np.ascontiguousarray(
            np.roll(co_matrix[SH * k : SH * (k + 1), :], shift, axis=1)
        )
        in_maps.append({"probs_r": p_r, "co_r": c_r})
    return in_maps


def finish(outs: np.ndarray) -> np.ndarray:
    """outs: [NCORES, 1, 2] per-core (partial_sum, partial_npos)."""
    total = np.float32(outs[:, 0, 0].astype(np.float64).sum())
    npos = np.float32(outs[:, 0, 1].astype(np.float64).sum())
    loss = (total / np.float32(B)) / (npos + np.float32(1e-8))
    return np.array(loss, dtype=np.float32)


TRACE = False
TRACE_DIR = None
LAST_RESULTS = None


def kernel(probs: np.ndarray, co_matrix: np.ndarray) -> np.ndarray:
    global LAST_RESULTS
    nc = _get_nc()
    in_maps = make_in_maps(probs, co_matrix)
    kwargs = {}
    if TRACE:
        kwargs = dict(trace=True, trace_cores=list(range(NCORES)), tmpdir=TRACE_DIR)
    res = run_bass_kernel_spmd(nc, in_maps, list(range(NCORES)), **kwargs)
    LAST_RESULTS = res
    outs = np.stack([r["out"] for r in res.results])
    return finish(outs)



# revision 9
# speedup vs baseline: 1.3795x; 1.3795x over previous
"""Trainium2 Bass kernel for the KB criterion loss.

Math
----
reference:
    diff[b,i,j] = probs[b,j] - probs[b,i]
    loss = sum_ij mean_b (diff^2 * C[i,j]) / (n_pos + 1e-8),  n_pos = count(C > 0)

Expanding the square removes the [B,N,N] intermediate entirely:

    sum_b (P[b,i] - P[b,j])^2 = S2_i + S2_j - 2*G_ij
        with S2_j = sum_b P[b,j]^2   and   G = P^T P  (Gram matrix)

so   total = sum_ij C_ij * D_ij,   D = S2_i + S2_j - 2 G_ij
     loss  = (total / B) / (n_pos + 1e-8)

Sharding (8 cores)
------------------
Shard C by rows: core k owns rows S_k = [128k, 128k+128). P is replicated.
Inputs are column-rolled by 128k so every core runs the same program with
its own row block mapped to local columns [0:128).

v2 (vs the 30.5us f32 baseline, trace-driven):
  * bf16 inputs (host downcast): halves DMA bytes AND turns the 16
    multi-pass fp32 HW matmuls (~12us cold) into 8 bf16 matmuls.
  * PE warmup: dummy matmuls on a zeroed tile during the DMA wait so the
    HAM clock gate reaches 8/8 (2.4 GHz) before the real matmuls issue.
  * tensor_tensor_reduce fuses (C * D') * -2 with the per-partition
    reduction in one DVE op per PSUM bank half; half 0 reduces while the
    PE still accumulates half 1.
  * P is DMA'd in two halves so squaring/S2 start ~0.35us earlier.

Per-core pipeline:
  1. DMA P half0, P half1, C (bf16).
  2. DVE: Psq = P*P per half.
  3. PE:  S2h = -(1/2) * ones^T @ Psq per half -> [1, N] PSUM.
  4. ACT: copy S2h PSUM->SBUF (bf16) per half.
  5. PE:  D' per 512-col bank: Gram + s2h_i x 1 + 1 x s2h_j  (= -D/2).
  6. DVE: tensor_tensor_reduce: (C * D') * -2, accum -> partials col h.
  7. ACT: Sign(C) accum -> partials col 2 (n_pos).
  8. PE:  ones^T reductions -> [1,2] + [1,1]; ACT copies; DMA [1,3] out.

Host sums the 8 partial triples (the scalar all-reduce) and finishes the
division.
"""

import numpy as np
import ml_dtypes

import concourse.bass as bass
import concourse.tile as tile
from concourse import mybir
from concourse.bass_utils import run_bass_kernel_spmd

B = 128
N = 1024
NCORES = 8
SH = N // NCORES  # 128 rows of C per core
F32 = mybir.dt.float32
BF16 = mybir.dt.bfloat16
HALF = 512  # PSUM bank width in fp32
WARMUP = 8  # dummy matmuls that warm the PE clock gate during DMA wait


def build_bass() -> bass.Bass:
    nc = bass.Bass()
    p_d = nc.dram_tensor("probs_r", [B, N], BF16, kind="ExternalInput")
    c_d = nc.dram_tensor("co_r", [SH, N], BF16, kind="ExternalInput")
    o_d = nc.dram_tensor("out", [1, 3], F32, kind="ExternalOutput")

    with tile.TileContext(nc) as tc:
        with (
            tc.tile_pool(name="sb", bufs=1) as sb,
            tc.tile_pool(name="ps", bufs=1, space="PSUM") as ps,
        ):
            p_sb = sb.tile([B, N], BF16)
            c_sb = sb.tile([SH, N], BF16)
            psq = sb.tile([B, N], BF16)
            s2h = sb.tile([1, N], BF16)
            ones_row = sb.tile([1, HALF], BF16)
            nh_col = sb.tile([B, 1], BF16)
            ones_col = sb.tile([B, 1], F32)
            dummy_sb = sb.tile([B, HALF], BF16)
            scr_mul = sb.tile([SH, N], BF16)
            scr_cnt = sb.tile([SH, N], BF16)
            partials = sb.tile([B, 4], F32)
            out_sb = sb.tile([1, 3], F32)

            d_ps = ps.tile([B, N], F32)  # banks 0-1
            s2_ps0 = ps.tile([1, HALF], F32)
            s2_ps1 = ps.tile([1, HALF], F32)
            fin = ps.tile([1, 3], F32)
            dummy_ps = ps.tile([B, HALF], F32)

            # Constants — all DVE-born: every matmul operand then traces to
            # a single upstream engine (trn2 LDWEIGHTS carries ONE sync-wait
            # slot). The whole kernel runs on PE + DVE + Sync; the scalar
            # engine (and its 1.3us ACT table load) is unused.
            nc.vector.memset(nh_col, -0.5)
            nc.vector.memset(ones_row, 1.0)
            nc.vector.memset(ones_col, 1.0)
            nc.vector.memset(dummy_sb, 0.0)

            # PE warmup: keep the PE busy ~3.4us so the HAM clock gate opens
            # to 8/8 before the real matmuls. Results are never read.
            for _ in range(WARMUP):
                nc.tensor.matmul(
                    dummy_ps, dummy_sb[:, 0:B], dummy_sb, start=True, stop=True
                )

            # Loads: P halves first (they head the critical path), then C.
            nc.sync.dma_start(out=p_sb[:, 0:HALF], in_=p_d[:, 0:HALF])
            nc.sync.dma_start(out=p_sb[:, HALF:N], in_=p_d[:, HALF:N])
            nc.sync.dma_start(out=c_sb, in_=c_d[:, :])

            # Psq = P*P per half
            nc.vector.tensor_mul(psq[:, 0:HALF], p_sb[:, 0:HALF], p_sb[:, 0:HALF])
            nc.vector.tensor_mul(psq[:, HALF:N], p_sb[:, HALF:N], p_sb[:, HALF:N])

            # -S2/2 = (-1/2)*colsum_b(Psq) -> [1, N] PSUM, then to SBUF bf16
            # (DVE copy: 2x faster than an ACT pass and keeps ACT retired)
            nc.tensor.matmul(s2_ps0, nh_col, psq[:, 0:HALF], start=True, stop=True)
            nc.tensor.matmul(s2_ps1, nh_col, psq[:, HALF:N], start=True, stop=True)
            nc.vector.tensor_copy(s2h[0:1, 0:HALF], s2_ps0)
            nc.vector.tensor_copy(s2h[0:1, HALF:N], s2_ps1)

            # n_pos per partition on DVE: sum_j (C > 0)
            nc.vector.tensor_scalar(
                scr_cnt, c_sb, 0.0, None, mybir.AluOpType.is_gt,
                mybir.AluOpType.add, accum_out=partials[:, 2:3],
            )

            # D' = G - S2_i/2 - S2_j/2 (= -D/2) accumulated per PSUM bank.
            # The fused DVE op computes (-2*D')*C = C*D elementwise AND the
            # per-partition row sum; half 0 reduces while the PE still
            # accumulates half 1 into the other bank.
            for h in range(2):
                js = slice(HALF * h, HALF * (h + 1))
                nc.tensor.matmul(
                    d_ps[:, js], p_sb[:, 0:SH], p_sb[:, js], start=True, stop=False
                )
                nc.tensor.matmul(
                    d_ps[:, js], s2h[0:1, 0:SH], ones_row[0:1, :],
                    start=False, stop=False,
                )
                nc.tensor.matmul(
                    d_ps[:, js], ones_row[0:1, 0:SH], s2h[0:1, js],
                    start=False, stop=True,
                )
                nc.vector.tensor_mul(scr_mul[:, js], c_sb[:, js], d_ps[:, js])
                # cheap row-reduce: tensor_scalar mult-by-1 with accum runs
                # at 4x for bf16 SBUF (~194ns/half vs 720ns for an ACT pass)
                nc.vector.tensor_scalar(
                    scr_cnt[:, js], scr_mul[:, js], 1.0, None,
                    mybir.AluOpType.mult, mybir.AluOpType.add,
                    accum_out=partials[:, h : h + 1],
                )

            # partition reduce -> scalars; every operand is DVE-produced.
            nc.tensor.matmul(fin, ones_col, partials[:, 0:3], start=True, stop=True)
            nc.vector.tensor_copy(out_sb[0:1, 0:3], fin)

            nc.sync.dma_start(out=o_d[:, :], in_=out_sb)

    _split_multi_waits(nc)
    return nc


def _split_multi_waits(nc: bass.Bass):
    """This walrus build accepts only ONE sync-wait per instruction
    ("Too many sync wait commands"). Tile's kernel-tail drain carries one
    wait per live semaphore; peel the extras onto same-engine NOPs that
    each stall on a single semaphore — semantically identical."""
    for bb in nc.main_func.blocks:
        insts = bb.instructions
        i = 0
        while i < len(insts):
            ins = insts[i]
            si = getattr(ins, "sync_info", None)
            if si is not None and si.on_wait is not None and len(si.on_wait) > 1:
                waits = list(si.on_wait)
                nops = []
                for j, w in enumerate(waits[:-1]):
                    nop = mybir.InstNoOp(
                        name=f"{ins.name}-wsplit{j}",
                        sync_info=mybir.SyncInfo(on_wait=[w], on_update=[]),
                        bass_nofuse=True,
                        engine=ins.engine,
                    )
                    nc.register_instruction(nop, overwrite=True)
                    nops.append(nop)
                si.on_wait = [waits[-1]]
                insts[i:i] = nops
                i += len(nops)
            i += 1


_NC = None


def _get_nc() -> bass.Bass:
    global _NC
    if _NC is None:
        _NC = build_bass()
    return _NC


def make_in_maps(probs: np.ndarray, co_matrix: np.ndarray):
    probs = np.ascontiguousarray(np.asarray(probs, dtype=np.float32))
    co_matrix = np.ascontiguousarray(np.asarray(co_matrix, dtype=np.float32))
    in_maps = []
    for k in range(NCORES):
        shift = -SH * k
        p_r = np.ascontiguousarray(
            np.roll(probs, shift, axis=1).astype(ml_dtypes.bfloat16)
        )
        c_r = np.ascontiguousarray(
            np.roll(co_matrix[SH * k : SH * (k + 1), :], shift, axis=1).astype(
                ml_dtypes.bfloat16
            )
        )
        in_maps.append({"probs_r": p_r, "co_r": c_r})
    return in_maps


def finish(outs: np.ndarray) -> np.ndarray:
    """outs: [NCORES, 1, 3] per-core (sum C*D' half0, half1, npos).

    D' = -D/2, so sum C*D = -2 * (col0 + col1)."""
    total = np.float32(
        -2.0
        * (outs[:, 0, 0].astype(np.float64) + outs[:, 0, 1].astype(np.float64)).sum()
    )
    npos = np.float32(outs[:, 0, 2].astype(np.float64).sum())
    loss = (total / np.float32(B)) / (npos + np.float32(1e-8))
    return np.array(loss, dtype=np.float32)


TRACE = False
TRACE_DIR = None
LAST_RESULTS = None


def kernel(probs: np.ndarray, co_matrix: np.ndarray) -> np.ndarray:
    global LAST_RESULTS
    nc = _get_nc()
    in_maps = make_in_maps(probs, co_matrix)
    kwargs = {}
    if TRACE:
        kwargs = dict(trace=True, trace_cores=list(range(NCORES)), tmpdir=TRACE_DIR)
    res = run_bass_kernel_spmd(nc, in_maps, list(range(NCORES)), **kwargs)
    LAST_RESULTS = res
    outs = np.stack([r["out"] for r in res.results])
    return finish(outs)


# revision 14
# speedup vs baseline: 1.4462x; 1.0483x over previous
"""Trainium2 Bass kernel for the KB criterion loss.

Math
----
reference:
    diff[b,i,j] = probs[b,j] - probs[b,i]
    loss = sum_ij mean_b (diff^2 * C[i,j]) / (n_pos + 1e-8),  n_pos = count(C > 0)

Expanding the square removes the [B,N,N] intermediate entirely:

    sum_b (P[b,i] - P[b,j])^2 = S2_i + S2_j - 2*G_ij
        with S2_j = sum_b P[b,j]^2   and   G = P^T P  (Gram matrix)

so   total = sum_ij C_ij * D_ij,   D = S2_i + S2_j - 2 G_ij
     loss  = (total / B) / (n_pos + 1e-8)

Sharding (8 cores)
------------------
Shard C by rows: core k owns rows S_k = [128k, 128k+128). P is replicated.
Inputs are column-rolled by 128k so every core runs the same program with
its own row block mapped to local columns [0:128).

v2 (vs the 30.5us f32 baseline, trace-driven):
  * bf16 inputs (host downcast): halves DMA bytes AND turns the 16
    multi-pass fp32 HW matmuls (~12us cold) into 8 bf16 matmuls.
  * PE warmup: dummy matmuls on a zeroed tile during the DMA wait so the
    HAM clock gate reaches 8/8 (2.4 GHz) before the real matmuls issue.
  * tensor_tensor_reduce fuses (C * D') * -2 with the per-partition
    reduction in one DVE op per PSUM bank half; half 0 reduces while the
    PE still accumulates half 1.
  * P is DMA'd in two halves so squaring/S2 start ~0.35us earlier.

Per-core pipeline:
  1. DMA P half0, P half1, C (bf16).
  2. DVE: Psq = P*P per half.
  3. PE:  S2h = -(1/2) * ones^T @ Psq per half -> [1, N] PSUM.
  4. ACT: copy S2h PSUM->SBUF (bf16) per half.
  5. PE:  D' per 512-col bank: Gram + s2h_i x 1 + 1 x s2h_j  (= -D/2).
  6. DVE: tensor_tensor_reduce: (C * D') * -2, accum -> partials col h.
  7. ACT: Sign(C) accum -> partials col 2 (n_pos).
  8. PE:  ones^T reductions -> [1,2] + [1,1]; ACT copies; DMA [1,3] out.

Host sums the 8 partial triples (the scalar all-reduce) and finishes the
division.
"""

import numpy as np
import ml_dtypes

import concourse.bass as bass
import concourse.tile as tile
from concourse import mybir
from concourse.bass_utils import run_bass_kernel_spmd

B = 128
N = 1024
NCORES = 8
SH = N // NCORES  # 128 rows of C per core
F32 = mybir.dt.float32
BF16 = mybir.dt.bfloat16
HALF = 512  # PSUM bank width in fp32
WARMUP = 6  # dummy matmuls that warm the PE clock gate during DMA wait


def build_bass() -> bass.Bass:
    nc = bass.Bass()
    p_d = nc.dram_tensor("probs_r", [B, N], BF16, kind="ExternalInput")
    c_d = nc.dram_tensor("co_r", [SH, N], BF16, kind="ExternalInput")
    o_d = nc.dram_tensor("out", [1, 3], F32, kind="ExternalOutput")

    with tile.TileContext(nc) as tc:
        with (
            tc.tile_pool(name="sb", bufs=1) as sb,
            tc.tile_pool(name="ps", bufs=1, space="PSUM") as ps,
        ):
            p_sb = sb.tile([B, N], BF16)
            c_sb = sb.tile([SH, N], BF16)
            psq = sb.tile([B, N], BF16)
            s2h = sb.tile([1, N], BF16)
            ones_row = sb.tile([1, HALF], BF16)
            nh_col = sb.tile([B, 1], BF16)
            ones_col = sb.tile([B, 1], F32)
            dummy_sb = sb.tile([B, HALF], BF16)
            scr_mul = sb.tile([SH, N], BF16)
            scr_cnt = sb.tile([SH, N], BF16)
            scr_red = sb.tile([SH, N], BF16)
            partials = sb.tile([B, 4], F32)
            out_sb = sb.tile([1, 3], F32)

            d_ps = ps.tile([B, N], F32)  # banks 0-1
            s2_ps0 = ps.tile([1, HALF], F32)
            s2_ps1 = ps.tile([1, HALF], F32)
            fin = ps.tile([1, 3], F32)
            dummy_ps = ps.tile([B, HALF], F32)

            # PE warmup: keep the PE busy until the real matmuls take over so
            # the HAM clock gate opens to 8/8 mid-kernel. The HAM watches
            # data activity, so the dummy operands must be NONZERO and varied
            # (an all-zeros warmup measured 0 HAM transitions) — fill via
            # gpsimd iota, which also keeps the warmup off the DVE critical
            # path. Results are never read.
            nc.gpsimd.iota(
                dummy_sb, [[1, HALF]], channel_multiplier=1,
                allow_small_or_imprecise_dtypes=True,
            )
            for _ in range(WARMUP):
                nc.tensor.matmul(
                    dummy_ps, dummy_sb[:, 0:B], dummy_sb, start=True, stop=True
                )

            # Constants. trn2 LDWEIGHTS carries ONE sync-wait slot, so each
            # matmul's operands should trace to a single upstream engine:
            # DVE-born consts pair with DVE-produced psq / partials.
            nc.vector.memset(nh_col, -0.5)
            nc.vector.memset(ones_row, 1.0)
            nc.vector.memset(ones_col, 1.0)

            # Loads: P halves first (they head the critical path), then C.
            nc.sync.dma_start(out=p_sb[:, 0:HALF], in_=p_d[:, 0:HALF])
            nc.sync.dma_start(out=p_sb[:, HALF:N], in_=p_d[:, HALF:N])
            nc.sync.dma_start(out=c_sb, in_=c_d[:, :])

            # Psq = P*P per half
            nc.vector.tensor_mul(psq[:, 0:HALF], p_sb[:, 0:HALF], p_sb[:, 0:HALF])
            nc.vector.tensor_mul(psq[:, HALF:N], p_sb[:, HALF:N], p_sb[:, HALF:N])

            # -S2/2 = (-1/2)*colsum_b(Psq) -> [1, N] PSUM, then to SBUF bf16.
            # The casts and the n_pos count run on the otherwise-idle ACT
            # engine: every DVE op with a PSUM operand or an accumulator
            # measured at 1x anyway, and DVE is the bottleneck engine.
            nc.tensor.matmul(s2_ps0, nh_col, psq[:, 0:HALF], start=True, stop=True)
            nc.tensor.matmul(s2_ps1, nh_col, psq[:, HALF:N], start=True, stop=True)
            nc.scalar.copy(s2h[0:1, 0:HALF], s2_ps0)
            nc.scalar.copy(s2h[0:1, HALF:N], s2_ps1)

            # n_pos per partition: sum_j sign(C)  (C >= 0 always)
            nc.scalar.activation(
                scr_cnt, c_sb, mybir.ActivationFunctionType.Sign,
                accum_out=partials[:, 2:3],
            )

            # D' = G - S2_i/2 - S2_j/2 (= -D/2) accumulated per PSUM bank.
            # The fused DVE op computes (-2*D')*C = C*D elementwise AND the
            # per-partition row sum; half 0 reduces while the PE still
            # accumulates half 1 into the other bank.
            for h in range(2):
                js = slice(HALF * h, HALF * (h + 1))
                nc.tensor.matmul(
                    d_ps[:, js], p_sb[:, 0:SH], p_sb[:, js], start=True, stop=False
                )
                nc.tensor.matmul(
                    d_ps[:, js], s2h[0:1, 0:SH], ones_row[0:1, :],
                    start=False, stop=False,
                )
                nc.tensor.matmul(
                    d_ps[:, js], ones_row[0:1, 0:SH], s2h[0:1, js],
                    start=False, stop=True,
                )
                nc.vector.tensor_mul(scr_mul[:, js], c_sb[:, js], d_ps[:, js])
                # cheap row-reduce: tensor_scalar mult-by-1 with accum runs
                # at 4x for bf16 SBUF (~194ns/half vs 720ns for an ACT pass)
                nc.vector.tensor_scalar(
                    scr_red[:, js], scr_mul[:, js], 1.0, None,
                    mybir.AluOpType.mult, mybir.AluOpType.add,
                    accum_out=partials[:, h : h + 1],
                )

            # partition reduce -> scalars; every operand is DVE-produced.
            nc.tensor.matmul(fin, ones_col, partials[:, 0:3], start=True, stop=True)
            nc.vector.tensor_copy(out_sb[0:1, 0:3], fin)

            nc.sync.dma_start(out=o_d[:, :], in_=out_sb)

    _split_multi_waits(nc)
    return nc


def _split_multi_waits(nc: bass.Bass):
    """This walrus build accepts only ONE sync-wait per instruction
    ("Too many sync wait commands"). Tile's kernel-tail drain carries one
    wait per live semaphore; peel the extras onto same-engine NOPs that
    each stall on a single semaphore — semantically identical."""
    for bb in nc.main_func.blocks:
        insts = bb.instructions
        i = 0
        while i < len(insts):
            ins = insts[i]
            si = getattr(ins, "sync_info", None)
            if si is not None and si.on_wait is not None and len(si.on_wait) > 1:
                waits = list(si.on_wait)
                nops = []
                for j, w in enumerate(waits[:-1]):
                    nop = mybir.InstNoOp(
                        name=f"{ins.name}-wsplit{j}",
                        sync_info=mybir.SyncInfo(on_wait=[w], on_update=[]),
                        bass_nofuse=True,
                        engine=ins.engine,
                    )
                    nc.register_instruction(nop, overwrite=True)
                    nops.append(nop)
                si.on_wait = [waits[-1]]
                insts[i:i] = nops
                i += len(nops)
            i += 1


_NC = None


def _get_nc() -> bass.Bass:
    global _NC
    if _NC is None:
        _NC = build_bass()
    return _NC


def make_in_maps(probs: np.ndarray, co_matrix: np.ndarray):
    probs = np.ascontiguousarray(np.asarray(probs, dtype=np.float32))
    co_matrix = np.ascontiguousarray(np.asarray(co_matrix, dtype=np.float32))
    in_maps = []
    for k in range(NCORES):
        shift = -SH * k
        p_r = np.ascontiguousarray(
            np.roll(probs, shift, axis=1).astype(ml_dtypes.bfloat16)
        )
        c_r = np.ascontiguousarray(
            np.roll(co_matrix[SH * k : SH * (k + 1), :], shift, axis=1).astype(
                ml_dtypes.bfloat16
            )
        )
        in_maps.append({"probs_r": p_r, "co_r": c_r})
    return in_maps


def finish(outs: np.ndarray) -> np.ndarray:
    """outs: [NCORES, 1, 3] per-core (sum C*D' half0, half1, npos).

    D' = -D/2, so sum C*D = -2 * (col0 + col1)."""
    total = np.float32(
        -2.0
        * (outs[:, 0, 0].astype(np.float64) + outs[:, 0, 1].astype(np.float64)).sum()
    )
    npos = np.float32(outs[:, 0, 2].astype(np.float64).sum())
    loss = (total / np.float32(B)) / (npos + np.float32(1e-8))
    return np.array(loss, dtype=np.float32)


TRACE = False
TRACE_DIR = None
LAST_RESULTS = None


def kernel(probs: np.ndarray, co_matrix: np.ndarray) -> np.ndarray:
    global LAST_RESULTS
    nc = _get_nc()
    in_maps = make_in_maps(probs, co_matrix)
    kwargs = {}
    if TRACE:
        kwargs = dict(trace=True, trace_cores=list(range(NCORES)), tmpdir=TRACE_DIR)
    res = run_bass_kernel_spmd(nc, in_maps, list(range(NCORES)), **kwargs)
    LAST_RESULTS = res
    outs = np.stack([r["out"] for r in res.results])
    return finish(outs)
